# revision 1
# baseline (speedup 1.0000x reference)
"""TRN2 Bass kernel for nn_LocalSelfAttn (LN -> packed QKV -> banded attention
(window +-16) -> out-proj -> residual), sharded 8-way over (B, T):
8 cores x 1024 tokens, each with a zero-padded 128-token halo strip.

Design (vs the original per-head S-orientation kernel):
  - scores computed transposed (S^T[w, q]) so exp output IS P^T: no PE
    transposes of P, no PSUM->SBUF copies in the attention inner loop
  - even/odd heads run as paired row-group (0,0)/(64,0) matmuls straight
    from qk_sb (concurrent in the PE array); head slots are parity-major
    because mixed row-groups must not share a PSUM bank
  - ones-column in v_sb gives softmax row-sums from the PV matmul for free;
    PV is q-stationary so 1/rowsum is a per-partition scalar (one
    broadcast multiply normalizes all heads)
  - normalized O is XBAR-DMA-transposed; out-proj runs fp8e4m3 with
    perf_mode=DoubleRow (256-deep contraction, 2 matmuls), weights
    pre-scaled x64 on host and de-scaled in the residual add
  - phase-A h^T via dma_start_transpose directly into ht_sb (no PE
    transposes); x is bf16 to halve input DMA
  - software pipelining: out-proj of block b-2 fills the PE while block
    b's exp/mask chain runs; DMAs spread over sync/scalar/gpsimd queues
    with x tiles first and W_V/W_out fetched late
"""

import sys

for _p in ("/opt/trn_rl_repo",):
    if _p not in sys.path:
        sys.path.insert(0, _p)

import numpy as np
import ml_dtypes

import concourse.bass as bass
import concourse.tile as tile
from concourse import bacc, mybir
from concourse.bass import ts
from concourse.bass_utils import run_bass_kernel_spmd

F32 = mybir.dt.float32
FP8 = mybir.dt.float8e4
BF16 = mybir.dt.bfloat16
AF = mybir.ActivationFunctionType
ALU = mybir.AluOpType

B, T, D, H, BAND = 2, 4096, 512, 8, 16
DH = D // H            # 64
LN_EPS = 1e-5
N_CORES = 8
PC = 1024              # tokens per core
HALO = 128
ST = PC + 2 * HALO     # strip tokens = 1280
NT = ST // 128         # 10 LN tiles
NB = PC // 128         # 8 query blocks
WIN = 160              # key window per 128-query block

SIG = [0, 2, 4, 6, 1, 3, 5, 7]        # head -> psum slot (parity-major)

_NC_CACHE = None


def build_bass():
    nc = bacc.Bacc(None, target_bir_lowering=False)
    xin = nc.declare_dram_parameter("xin", [ST, D], BF16, isOutput=False)
    wqk = nc.declare_dram_parameter("wqk", [4, 128, 2 * D], BF16, isOutput=False)
    wv = nc.declare_dram_parameter("wv", [4, 128, D], BF16, isOutput=False)
    wout8 = nc.declare_dram_parameter("wout8", [2, 128, 2, D], FP8, isOutput=False)
    beffqk = nc.declare_dram_parameter("beffqk", [128, 8], F32, isOutput=False)
    bvrep = nc.declare_dram_parameter("bvrep", [128, D], BF16, isOutput=False)
    borep = nc.declare_dram_parameter("borep", [128, D], BF16, isOutput=False)
    bmaskh = nc.declare_dram_parameter("bmaskh", [128, NB, 128], BF16, isOutput=False)
    bmaskl = nc.declare_dram_parameter("bmaskl", [32, NB, 32], BF16, isOutput=False)
    yout = nc.declare_dram_parameter("yout", [PC, D], F32, isOutput=True)

    with tile.TileContext(nc) as tc:
        from contextlib import ExitStack

        with ExitStack() as ctx:
            const = ctx.enter_context(tc.tile_pool(name="const", bufs=1))
            sb = ctx.enter_context(tc.tile_pool(name="sb", bufs=1))
            ln = ctx.enter_context(tc.tile_pool(name="ln", bufs=4))
            cp = ctx.enter_context(tc.tile_pool(name="cp", bufs=3))
            at = ctx.enter_context(tc.tile_pool(name="at", bufs=3))
            osTp = ctx.enter_context(tc.tile_pool(name="osTp", bufs=3))

            # ---- x tiles first on sync queue (DMA priority) ----
            x_sb = sb.tile([128, NT, D], BF16)         # input tiles (residual)
            TORD = [1, 2, 3, 4, 0, 5, 6, 7, 8, 9]
            for t in TORD:
                nc.sync.dma_start(out=x_sb[:, t, :], in_=xin[ts(t, 128), :])

            # ---- constants (spread across DMA queues; W_V / W_out later) ----
            beff_sb = const.tile([128, 8], F32)
            nc.scalar.dma_start(out=beff_sb, in_=beffqk[:, :])
            w_sb = const.tile([128, 4, 3 * D], BF16)
            nc.scalar.dma_start(out=w_sb[:, :, 0:2 * D],
                                in_=wqk.rearrange("c p j -> p c j"))
            wo_sb = const.tile([128, 2, 2, D], FP8)
            bv_sb = const.tile([128, 8, DH], BF16)
            nc.gpsimd.dma_start(out=bv_sb, in_=bvrep[:, :])
            bo_sb = const.tile([128, D], BF16)
            nc.gpsimd.dma_start(out=bo_sb, in_=borep[:, :])
            bmh_sb = const.tile([128, NB, 128], BF16)
            nc.gpsimd.dma_start(out=bmh_sb, in_=bmaskh[:, :, :])
            bml_sb = const.tile([32, NB, 32], BF16)
            nc.gpsimd.dma_start(out=bml_sb, in_=bmaskl[:, :, :])
            eps_sb = const.tile([128, 1], F32)
            nc.vector.memset(eps_sb, LN_EPS)

            # ---- persistent activations ----
            ht_sb = sb.tile([128, 4, ST], BF16)        # h^T: [D(4x128), tok]
            qk_sb = sb.tile([128, 8, ST], BF16)        # qk^T: [1024(8x128), tok]
            v_sb = sb.tile([128, 9, H, DH + 1], BF16)  # v token-major + ones col
            nc.vector.memset(v_sb[:, :, :, DH:DH + 1], 1.0)
            ptlo_bufs = [sb.tile([32, 8, 128], BF16, name=f"ptlo{i}")
                         for i in range(2)]
            for pl in ptlo_bufs:
                nc.vector.memset(pl, 0.0)

            # ================= Phase A: LN + h^T + QKV =================
            with tc.tile_pool(name="psA", bufs=2, space="PSUM") as psA:
                for t in TORD:
                    stats = ln.tile([128, 6], F32)
                    nc.vector.bn_stats(out=stats, in_=x_sb[:, t, :])
                    mv = ln.tile([128, 2], F32)
                    nc.vector.bn_aggr(out=mv, in_=stats)
                    std = ln.tile([128, 1], F32)
                    nc.scalar.activation(out=std, in_=mv[:, 1:2], func=AF.Sqrt,
                                         bias=eps_sb)
                    rstd = ln.tile([128, 1], F32)
                    nc.vector.reciprocal(out=rstd, in_=std)
                    nbias = ln.tile([128, 1], F32)
                    nc.vector.tensor_scalar(
                        out=nbias, in0=mv[:, 0:1], scalar1=rstd, scalar2=-1.0,
                        op0=ALU.mult, op1=ALU.mult)
                    hbf = cp.tile([128, D], BF16)
                    nc.scalar.activation(out=hbf, in_=x_sb[:, t, :], func=AF.Identity,
                                         bias=nbias, scale=rstd)
                    eng = nc.sync if t % 2 == 0 else nc.scalar
                    eng.dma_start_transpose(ht_sb[:, :, ts(t, 128)], hbf)

                nc.scalar.dma_start(out=w_sb[:, :, 2 * D:3 * D],
                                    in_=wv.rearrange("c p j -> p c j"))
                nc.scalar.dma_start(out=wo_sb,
                                    in_=wout8.rearrange("c p k j -> p c k j"))

                # Q,K projection (transposed layout), trimmed token ranges
                #  Q rows (jc 0-3): tokens [128, 1152);  K rows (jc 4-7): [112, 1168)
                for jc in range(8):
                    qp = psA.tile([128, 1056], F32, tag="qk")
                    if jc < 4:
                        t_lo, chunks = 128, [(128, 640), (640, 1152)]
                    else:
                        t_lo, chunks = 112, [(112, 624), (624, 1136), (1136, 1168)]
                    for ic in range(4):
                        for (s0, s1) in chunks:
                            nc.tensor.matmul(
                                qp[:, s0 - t_lo:s1 - t_lo],
                                lhsT=w_sb[:, ic, ts(jc, 128)],
                                rhs=ht_sb[:, ic, s0:s1],
                                start=(ic == 0), stop=(ic == 3))
                    width = chunks[-1][1] - t_lo
                    nc.vector.tensor_scalar_add(
                        out=qk_sb[:, jc, t_lo:t_lo + width], in0=qp[:, 0:width],
                        scalar1=beff_sb[:, jc:jc + 1])

                # V projection token-major (tokens [112, 1168)), + bias, + 65-stride
                for vt in range(9):
                    base = 112 + 128 * vt
                    rows = 128 if vt < 8 else 32
                    vp = psA.tile([128, D], F32, tag="v")
                    for ic in range(4):
                        nc.tensor.matmul(
                            vp[0:rows, :], lhsT=ht_sb[:, ic, base:base + rows],
                            rhs=w_sb[:, ic, 2 * D:3 * D],
                            start=(ic == 0), stop=(ic == 3))
                    nc.vector.tensor_add(
                        out=v_sb[0:rows, vt, :, 0:DH],
                        in0=vp[0:rows, :], in1=bv_sb[0:rows, :, :])

            # ================= Phase B: attention blocks =================
            with (
                tc.tile_pool(name="psS", bufs=1, space="PSUM") as psS,
                tc.tile_pool(name="psO", bufs=1, space="PSUM") as psO,
                tc.tile_pool(name="psY", bufs=1, space="PSUM") as psY,
            ):
                HORD = [0, 2, 4, 6, 1, 3, 5, 7]   # evens first, then odds
                osT_ring = {}

                def emit_outproj(b, osT8):
                    yp = psY.tile([128, D], F32, tag="yp", name="yp")
                    for c in range(2):
                        nc.tensor.matmul(
                            yp, lhsT=osT8[:, 2 * c:2 * c + 2, :],
                            rhs=wo_sb[:, c, :, :], start=(c == 0), stop=(c == 1),
                            perf_mode=mybir.MatmulPerfMode.DoubleRow)
                    ysb = at.tile([128, D], F32, tag="ysb", name="ysb")
                    nc.vector.scalar_tensor_tensor(
                        out=ysb, in0=yp, scalar=1.0 / 64.0, in1=x_sb[:, b + 1, :],
                        op0=ALU.mult, op1=ALU.add)
                    nc.vector.tensor_add(out=ysb, in0=ysb, in1=bo_sb)
                    nc.gpsimd.dma_start(out=yout[ts(b, 128), :], in_=ysb)

                for b in range(NB):
                    q0 = 128 * (b + 1)
                    k0 = q0 - BAND
                    # --- scores S^T: [w, q]; slot order SIG: evens banks 0-1,
                    #     odds banks 2-3 (mixed row-groups must not share a bank)
                    # paired emission: even/odd heads run concurrently in row
                    # groups (0,0)/(64,0); adjacent in-stream for overlap
                    sT = psS.tile([128, 8, 256], F32, tag="sT")
                    for qc in range(4):
                        for part in ("hi", "lo"):
                            for h in (2 * qc, 2 * qc + 1):
                                s = SIG[h]
                                p64 = 64 * (h % 2)
                                if part == "hi":
                                    nc.tensor.matmul(
                                        sT[:, s, 0:128],
                                        lhsT=qk_sb[p64:p64 + 64, 4 + qc,
                                                   k0:k0 + 128],
                                        rhs=qk_sb[p64:p64 + 64, qc, q0:q0 + 128],
                                        start=True, stop=True)
                                else:
                                    nc.tensor.matmul(
                                        sT[0:32, s, 224:256],
                                        lhsT=qk_sb[p64:p64 + 64, 4 + qc,
                                                   k0 + 128:k0 + 160],
                                        rhs=qk_sb[p64:p64 + 64, qc,
                                                  q0 + 96:q0 + 128],
                                        start=True, stop=True)
                    # --- exp -> P^T (bf16) + mask multiply, split by parity so
                    #     even-head PV can start after half the chain
                    pth = at.tile([128, 8, 128], BF16, tag="pth")
                    ptl = ptlo_bufs[b % 2]
                    for g in range(2):
                        sl = slice(4 * g, 4 * g + 4)
                        nc.scalar.activation(out=pth[:, sl, :],
                                             in_=sT[:, sl, 0:128], func=AF.Exp)
                        nc.scalar.activation(out=ptl[:, sl, 96:128],
                                             in_=sT[0:32, sl, 224:256], func=AF.Exp)
                        nc.vector.tensor_mul(
                            out=pth[:, sl, :], in0=pth[:, sl, :],
                            in1=bmh_sb[:, b:b + 1, :].to_broadcast([128, 4, 128]))
                        nc.vector.tensor_mul(
                            out=ptl[:, sl, 96:128], in0=ptl[:, sl, 96:128],
                            in1=bml_sb[:, b:b + 1, :].to_broadcast([32, 4, 32]))
                    # --- out-proj of block b-2: its osT transpose had a full
                    #     block cycle to finish, so the PE never waits on it
                    if b >= 2:
                        emit_outproj(b - 2, osT_ring.pop(b - 2))
                    # --- PV (q-stationary): ov[q, h, 0:64]=O, [..,64]=rowsum
                    ov = psO.tile([128, 8, 128], F32, tag="ov")
                    for h in HORD:
                        s = SIG[h]
                        nc.tensor.matmul(
                            ov[:, h, 0:DH + 1],
                            lhsT=pth[:, s, :], rhs=v_sb[:, b, h, :],
                            start=True, stop=False)
                        nc.tensor.matmul(
                            ov[:, h, 0:DH + 1],
                            lhsT=ptl[0:32, s, :], rhs=v_sb[0:32, b + 1, h, :],
                            start=False, stop=True)
                    # --- normalize -> osb_q [q, h*64] bf16 (one broadcast mult)
                    rsi = at.tile([128, 8, 1], F32, tag="rsi")
                    nc.vector.reciprocal(out=rsi, in_=ov[:, :, DH:DH + 1])
                    osq = at.tile([128, 8, DH], BF16, tag="osq")
                    nc.vector.tensor_mul(
                        out=osq, in0=ov[:, :, 0:DH],
                        in1=rsi.to_broadcast([128, 8, DH]))
                    # --- transpose O via XBAR -> [d(4x128), q]
                    osT = osTp.tile([128, 4, 128], BF16, tag="osT")
                    nc.sync.dma_start_transpose(osT, osq)
                    osT8 = osTp.tile([128, 4, 128], FP8, tag="osT8")
                    nc.vector.tensor_copy(osT8, osT)
                    osT_ring[b] = osT8
                    if b == NB - 1:
                        emit_outproj(NB - 2, osT_ring.pop(NB - 2))
                emit_outproj(NB - 1, osT_ring.pop(NB - 1))
    nc.finalize()
    return nc


def make_in_maps(x, ln_g, ln_b, w_in, b_in, w_out, b_out):
    x = np.asarray(x, np.float32)
    ln_g = np.asarray(ln_g, np.float32)
    ln_b = np.asarray(ln_b, np.float32)
    w_in = np.asarray(w_in, np.float32)
    b_in = np.asarray(b_in, np.float32)
    w_out = np.asarray(w_out, np.float32)
    b_out = np.asarray(b_out, np.float32)

    # fold LN affine + 1/sqrt(dh) into the packed projection
    w_eff = w_in * ln_g[None, :]
    b_eff = b_in + w_in @ ln_b
    sc = np.float32(1.0 / np.sqrt(DH))
    w_eff = w_eff.copy()
    w_eff[:D] *= sc
    b_eff = b_eff.copy()
    b_eff[:D] *= sc

    bf = ml_dtypes.bfloat16
    wallT = np.ascontiguousarray(w_eff.T.reshape(4, 128, 3 * D)).astype(bf)
    f8 = ml_dtypes.float8_e4m3
    wout8 = np.ascontiguousarray(
        (w_out.T * 64.0).reshape(2, 2, 128, D).transpose(0, 2, 1, 3)).astype(f8)
    beffqk = np.ascontiguousarray(b_eff[:2 * D].reshape(8, 128).T).astype(np.float32)
    bvrep = np.broadcast_to(b_eff[2 * D:].astype(bf), (128, D)).copy()
    borep = np.broadcast_to(b_out.astype(bf), (128, D)).copy()

    # masks in S^T [w, q] layout; hi: w 0-127 / q 0-127, lo: w 128-159 / q 96-127
    ww = np.arange(WIN)[:, None]
    qq = np.arange(128)[None, :]
    band = (ww - qq >= 0) & (ww - qq <= 2 * BAND)

    in_maps = []
    for c in range(N_CORES):
        batch = c // 4
        t0 = (c % 4) * PC
        xloc = np.zeros((ST, D), bf)
        lo = t0 - HALO
        s0, s1 = max(lo, 0), min(t0 + PC + HALO, T)
        xloc[s0 - lo:s1 - lo] = x[batch, s0:s1].astype(bf)
        bmh = np.zeros((NB, 128, 128), np.float32)
        bml = np.zeros((NB, 32, 32), np.float32)
        for b in range(NB):
            gk = t0 + 128 * b + 112 - HALO + ww  # global key index of window row w
            m = band & (gk >= 0) & (gk < T)
            bmh[b] = m[0:128, :]
            bml[b] = m[128:160, 96:128]
        in_maps.append(dict(
            xin=xloc, wqk=wallT[:, :, 0:2 * D], wv=wallT[:, :, 2 * D:3 * D],
            wout8=wout8, beffqk=beffqk,
            bvrep=bvrep, borep=borep,
            bmaskh=np.ascontiguousarray(bmh.transpose(1, 0, 2)).astype(bf),
            bmaskl=np.ascontiguousarray(bml.transpose(1, 0, 2)).astype(bf)))
    return in_maps


def kernel_run(inputs, trace=False, trace_kwargs=None):
    global _NC_CACHE
    if _NC_CACHE is None:
        _NC_CACHE = build_bass()
    nc = _NC_CACHE
    in_maps = make_in_maps(**inputs)
    kw = {}
    if trace:
        kw = dict(trace=True, trace_cores=[0], **(trace_kwargs or {}))
    res = run_bass_kernel_spmd(nc, in_maps, list(range(N_CORES)), **kw)
    y = np.stack([res.results[c]["yout"] for c in range(N_CORES)])
    out = y.reshape(B, T, D).astype(np.float32)
    return out, res


def kernel(**inputs):
    out, _ = kernel_run(inputs, trace=False)
    return out


if __name__ == "__main__":
    rng = np.random.default_rng(0)
    ins = dict(
        x=rng.standard_normal((B, T, D)).astype(np.float32),
        ln_g=np.ones(D, np.float32), ln_b=np.zeros(D, np.float32),
        w_in=(rng.standard_normal((3 * D, D)) * 0.02).astype(np.float32),
        b_in=np.zeros(3 * D, np.float32),
        w_out=(rng.standard_normal((D, D)) * 0.02).astype(np.float32),
        b_out=np.zeros(D, np.float32))
    out = kernel(**ins)
    print("ran:", out.shape, out.dtype)



# revision 6
# speedup vs baseline: 1.0775x; 1.0775x over previous
"""TRN2 Bass kernel for nn_LocalSelfAttn (LN -> packed QKV -> banded attention
(window +-16) -> out-proj -> residual), sharded 8-way over (B, T):
8 cores x 1024 tokens, 16-token halo strips (zero-padded at sequence edges).

v2 design (vs the 122us baseline):
  - x strip loaded in 3 batched DMAs (HWDGE descriptor-gen is a serial
    ~630ns/instr resource; the old 10 per-tile DMAs trickled in over 16us)
  - PE p-state warmup: a chain of tiny matmuls during LN keeps the PE
    continuously busy so QKV runs at 2.4GHz instead of 1.2GHz
  - QKV projection in fp8e4m3 with perf_mode=DoubleRow (256-deep
    contraction, 0.5 cyc/row): weights prescaled x64 on host, descale
    folded into the exp() scale and the out-proj epilogue
  - ht transposed via 3 batched XBAR DMAs (bf16), then cast to fp8 flat
    layout [d_chunk, tok] so matmul operands can slice arbitrary token
    windows
  - attention in 11 blocks of 96 queries x 128-key windows: no hi/lo
    split matmuls; one exp + one mask-mul per block
  - PV via fp8 DoubleRow pairing two heads per matmul (zero-padded V
    slabs); ones-columns give per-head softmax rowsums for free
  - scores pairs run even/odd head row-groups concurrently; sT PSUM is
    double-buffered (2 banks each) so next-block scores overlap exp
  - out-proj (fp8 DR) emitted as soon as its 128-query osT columns are
    transposed+cast; residual fetched separately from DRAM in f32
    (improves accuracy; off critical path on the gpsimd SWDGE queue)
"""

import sys

for _p in ("/opt/trn_rl_repo",):
    if _p not in sys.path:
        sys.path.insert(0, _p)

import numpy as np
import ml_dtypes

import concourse.bass as bass
import concourse.tile as tile
from concourse import bacc, mybir
from concourse.bass import ts
from concourse.bass_utils import run_bass_kernel_spmd

F32 = mybir.dt.float32
FP8 = mybir.dt.float8e4
BF16 = mybir.dt.bfloat16
AF = mybir.ActivationFunctionType
ALU = mybir.AluOpType
DR = mybir.MatmulPerfMode.DoubleRow

B, T, D, H, BAND = 2, 4096, 512, 8, 16
DH = D // H            # 64
LN_EPS = 1e-5
N_CORES = 8
PC = 1024              # tokens per core
HALO = 16
ST = PC + 2 * HALO     # strip = 1056 real tokens
STP = 1152             # strip padded to 9 full LN tiles
NT = 9                 # LN tiles
QB = 96                # queries per attention block
NBQ = 11               # 10 x 96 + 1 x 64
WIN = 128              # key window per block
NOB = 8                # out-proj blocks of 128 queries
W8SC = 64.0            # fp8 weight prescale
EXPSC = 1.0 / (W8SC * W8SC * np.sqrt(DH))   # descale both x64 + 1/sqrt(dh)
NWARM = 28

# out-proj block ob -> first attention block b whose osT cast completes
# columns [128*ob, 128*ob+128)
OB_DEADLINE = {0: 1, 1: 2, 2: 3, 3: 5, 4: 6, 5: 7, 6: 9, 7: 10}


def w0_of(b):
    return 96 * b if b < 10 else 928


_NC_CACHE = None


def build_bass():
    nc = bacc.Bacc(None, target_bir_lowering=False)
    xin = nc.declare_dram_parameter("xin", [STP, D], BF16, isOutput=False)
    xres = nc.declare_dram_parameter("xres", [PC, D], F32, isOutput=False)
    w8qk = nc.declare_dram_parameter("w8qk", [128, 2, 2, 2 * D], FP8, isOutput=False)
    w8v = nc.declare_dram_parameter("w8v", [128, 2, 2, D], FP8, isOutput=False)
    wo8 = nc.declare_dram_parameter("wo8", [2, 128, 2, D], FP8, isOutput=False)
    beffqk = nc.declare_dram_parameter("beffqk", [128, 8], F32, isOutput=False)
    bmask = nc.declare_dram_parameter("bmask", [128, NBQ, QB], FP8, isOutput=False)
    yout = nc.declare_dram_parameter("yout", [PC, D], F32, isOutput=True)

    with tile.TileContext(nc) as tc:
        from contextlib import ExitStack

        with ExitStack() as ctx:
            const = ctx.enter_context(tc.tile_pool(name="const", bufs=1))
            sb = ctx.enter_context(tc.tile_pool(name="sb", bufs=1))
            ln = ctx.enter_context(tc.tile_pool(name="ln", bufs=4))
            cpq = ctx.enter_context(tc.tile_pool(name="cpq", bufs=2))
            at = ctx.enter_context(tc.tile_pool(name="at", bufs=3))
            osTp = ctx.enter_context(tc.tile_pool(name="osTp", bufs=3))

            # ---- x strip in 3 batched DMAs on sync queue ----
            x_sb = sb.tile([128, NT, D], BF16)
            for g in range(3):
                nc.sync.dma_start(
                    out=x_sb[:, 3 * g:3 * g + 3, :],
                    in_=xin[384 * g:384 * g + 384, :].rearrange(
                        "(t p) d -> p t d", p=128))

            # ---- constants (weights on scalar queue; masks/xres on gpsimd) ----
            beff_sb = const.tile([128, 8], F32)
            nc.scalar.dma_start(out=beff_sb, in_=beffqk[:, :])
            w8_sb = const.tile([128, 2, 2, 2 * D], FP8)
            nc.scalar.dma_start(out=w8_sb, in_=w8qk[:, :, :, :])
            w8v_sb = const.tile([128, 2, 2, D], FP8)
            nc.scalar.dma_start(out=w8v_sb, in_=w8v[:, :, :, :])
            wo_sb = const.tile([128, 2, 2, D], FP8)
            nc.scalar.dma_start(out=wo_sb, in_=wo8.rearrange("c p k j -> p c k j"))
            bm_sb = const.tile([128, NBQ, 1, QB], FP8)
            nc.gpsimd.dma_start(out=bm_sb[:, :, 0, :], in_=bmask[:, :, :])
            xres_sb = const.tile([128, NOB, D], F32)
            for g in range(2):
                nc.gpsimd.dma_start(
                    out=xres_sb[:, 4 * g:4 * g + 4, :],
                    in_=xres[512 * g:512 * g + 512, :].rearrange(
                        "(t p) d -> p t d", p=128))
            eps_sb = const.tile([128, 1], F32)
            nc.vector.memset(eps_sb, LN_EPS)

            # ---- persistent activations ----
            hbf = sb.tile([128, NT, D], BF16)          # normalized h, token-major
            htst = sb.tile([128, 36, 128], BF16)       # h^T staging (tile,dc)-major
            ht8 = sb.tile([128, 4, STP], FP8)          # h^T fp8 flat [dc, tok]
            qk_sb = sb.tile([128, 8, 1088], BF16)      # q,k rows x64, flat tok
            nc.vector.memset(qk_sb[:, 0:4, 1040:1088], 0.0)  # block-10 query pad
            v8 = sb.tile([128, NBQ, 4, 2, 132], FP8)   # per-block DR V slabs
            nc.vector.memset(v8, 0.0)
            nc.vector.memset(v8[:, :, :, 0, 64:65], 1.0)   # rowsum ones cols
            nc.vector.memset(v8[:, :, :, 1, 130:131], 1.0)
            osT8 = sb.tile([128, 4, PC], FP8)          # O^T fp8 [dc, q]

            # ================= Phase A =================
            # PE p-state warmup: chained tiny matmuls during LN keep the PE
            # continuously busy so the first real matmuls run at full clock
            with tc.tile_pool(name="psW", bufs=1, space="PSUM") as psW:
                wt = psW.tile([128, 256], F32)
                for _ in range(NWARM):
                    nc.tensor.matmul(wt[0:1, :], lhsT=x_sb[:, 0, 0:1],
                                     rhs=x_sb[:, 0, 0:256], start=True, stop=True)

            # LayerNorm: stats on vector, sqrt on scalar, normalize split
            for t in range(NT):
                stats = ln.tile([128, 6], F32)
                nc.vector.bn_stats(out=stats, in_=x_sb[:, t, :])
                mv = ln.tile([128, 2], F32)
                nc.vector.bn_aggr(out=mv, in_=stats)
                std = ln.tile([128, 1], F32)
                nc.scalar.activation(out=std, in_=mv[:, 1:2], func=AF.Sqrt,
                                     bias=eps_sb)
                rstd = ln.tile([128, 1], F32)
                nc.vector.reciprocal(out=rstd, in_=std)
                nbias = ln.tile([128, 1], F32)
                nc.vector.tensor_scalar(
                    out=nbias, in0=mv[:, 0:1], scalar1=rstd, scalar2=-1.0,
                    op0=ALU.mult, op1=ALU.mult)
                if t % 2 == 0:
                    nc.scalar.activation(out=hbf[:, t, :], in_=x_sb[:, t, :],
                                         func=AF.Identity, bias=nbias, scale=rstd)
                else:
                    nc.vector.tensor_scalar(
                        out=hbf[:, t, :], in0=x_sb[:, t, :], scalar1=rstd,
                        scalar2=nbias, op0=ALU.mult, op1=ALU.add)
                if t % 3 == 2:
                    g = t // 3
                    nc.sync.dma_start_transpose(
                        htst[:, 12 * g:12 * g + 12, :], hbf[:, 3 * g:3 * g + 3, :])
                    # cast to fp8 + permute (tile,dc)-major -> flat [dc, tok]
                    nc.gpsimd.tensor_copy(
                        out=ht8[:, :, 384 * g:384 * g + 384].rearrange(
                            "p c (t x) -> p t c x", x=128),
                        in_=htst[:, 12 * g:12 * g + 12, :].rearrange(
                            "p (t c) x -> p t c x", c=4))

            with tc.tile_pool(name="psA", bufs=2, space="PSUM") as psA:
                # V projection for per-block 128-token key windows (fp8 DR).
                # Emits the DR rhs slabs [w, 2, 132] with head pairs zero-
                # padded; data cols [66k, 66k+64), ones col at 66k+64.
                def emit_vproj(b):
                    w0 = w0_of(b)
                    vp = psA.tile([128, 4, 2, 64], F32, tag="v", name="vp")
                    for c in range(2):
                        nc.tensor.matmul(
                            vp[:, :, :, :], lhsT=ht8[:, 2 * c:2 * c + 2, w0:w0 + 128],
                            rhs=w8v_sb[:, c, :, :], start=(c == 0), stop=(c == 1),
                            perf_mode=DR)
                    nc.vector.tensor_copy(
                        out=v8[:, b, :, 0, 0:64], in_=vp[:, :, 0, :])
                    nc.scalar.activation(
                        out=v8[:, b, :, 1, 66:130], in_=vp[:, :, 1, :],
                        func=AF.Identity)

                emit_vproj(0)
                emit_vproj(1)

                # Q (jc 0-3, tokens [16,1040)) and K (jc 4-7, tokens [0,1056))
                QCH = [(16, 528), (528, 1040)]
                KCH = [(0, 512), (512, 1024), (1024, 1056)]
                vnext = 2
                for jc in range(8):
                    t_lo, chunks = (16, QCH) if jc < 4 else (0, KCH)
                    qp = psA.tile([128, 1056], F32, tag="qk", name="qp")
                    for (s0, s1) in chunks:
                        for c in range(2):
                            nc.tensor.matmul(
                                qp[:, s0 - t_lo:s1 - t_lo],
                                lhsT=w8_sb[:, c, :, ts(jc, 128)],
                                rhs=ht8[:, 2 * c:2 * c + 2, s0:s1],
                                start=(c == 0), stop=(c == 1), perf_mode=DR)
                    if vnext < NBQ:
                        emit_vproj(vnext)
                        vnext += 1
                    width = chunks[-1][1] - t_lo
                    # PSUM -> SBUF bf16 with (x64-prescaled) bias add
                    if jc % 2 == 0:
                        nc.scalar.activation(
                            out=qk_sb[:, jc, t_lo:t_lo + width],
                            in_=qp[:, 0:width], func=AF.Identity,
                            bias=beff_sb[:, jc:jc + 1])
                    else:
                        nc.vector.tensor_scalar_add(
                            out=qk_sb[:, jc, t_lo:t_lo + width],
                            in0=qp[:, 0:width], scalar1=beff_sb[:, jc:jc + 1])
                while vnext < NBQ:
                    emit_vproj(vnext)
                    vnext += 1

            # ================= Phase B: attention =================
            with (
                tc.tile_pool(name="psS", bufs=2, space="PSUM") as psS,
                tc.tile_pool(name="psO", bufs=1, space="PSUM") as psO,
                tc.tile_pool(name="psY", bufs=2, space="PSUM") as psY,
            ):
                def emit_outproj(ob):
                    yp = psY.tile([128, D], F32, tag="yp", name="yp")
                    for c in range(2):
                        nc.tensor.matmul(
                            yp, lhsT=osT8[:, 2 * c:2 * c + 2, ts(ob, 128)],
                            rhs=wo_sb[:, c, :, :], start=(c == 0), stop=(c == 1),
                            perf_mode=DR)
                    ysb = at.tile([128, D], F32, tag="ysb", name="ysb")
                    nc.vector.scalar_tensor_tensor(
                        out=ysb, in0=yp, scalar=1.0 / W8SC,
                        in1=xres_sb[:, ob, :], op0=ALU.mult, op1=ALU.add)
                    nc.gpsimd.dma_start(out=yout[ts(ob, 128), :], in_=ysb)

                emit_at = {dl + 1: ob for ob, dl in OB_DEADLINE.items() if dl < 10}

                for b in range(NBQ):
                    q0 = HALO + QB * b
                    w0 = w0_of(b)
                    nq = QB if b < 10 else 64
                    # scores S^T[w, q]; head 2i+k -> slot 4k+i; even/odd head
                    # row-groups (partitions 0-63 / 64-127) run concurrently
                    # and land in different PSUM banks
                    sT = psS.tile([128, 8, 128], F32, tag="sT")
                    for i in range(4):
                        for k in range(2):
                            p64 = 64 * k
                            nc.tensor.matmul(
                                sT[:, 4 * k + i, 0:QB],
                                lhsT=qk_sb[p64:p64 + 64, 4 + i, w0:w0 + WIN],
                                rhs=qk_sb[p64:p64 + 64, i, q0:q0 + QB],
                                start=True, stop=True)
                    # exp (with all descales folded into scale) -> fp8 P
                    pth = at.tile([128, 4, 2, QB], FP8, tag="pth")
                    nc.scalar.activation(
                        out=pth[:, :, :, :].rearrange("p i k q -> p k i q"),
                        in_=sT[:, :, 0:QB].rearrange("p (k i) q -> p k i q", k=2),
                        func=AF.Exp, scale=float(EXPSC))
                    nc.vector.tensor_mul(
                        out=pth[:, :, :, :], in0=pth[:, :, :, :],
                        in1=bm_sb[:, b:b + 1, :, :].to_broadcast([128, 4, 2, QB]))
                    if b in emit_at:
                        emit_outproj(emit_at[b])
                    # PV: fp8 DR, two heads per matmul via zero-padded slabs
                    ova = psO.tile([128, 3, 2, 66], F32, tag="ova")
                    ovb = psO.tile([128, 1, 2, 66], F32, tag="ovb")
                    for i in range(4):
                        out = ova[0:QB, i, :, :] if i < 3 else ovb[0:QB, 0, :, :]
                        nc.tensor.matmul(
                            out, lhsT=pth[:, i, :, :], rhs=v8[:, b, i, :, :],
                            start=True, stop=True, perf_mode=DR)
                    # normalize: rowsums at col 64 of each slab
                    rsa = at.tile([128, 3, 2, 1], F32, tag="rsa")
                    nc.vector.reciprocal(out=rsa[0:QB], in_=ova[0:QB, :, :, 64:65])
                    rsb = at.tile([128, 1, 2, 1], F32, tag="rsb")
                    nc.vector.reciprocal(out=rsb[0:QB], in_=ovb[0:QB, :, :, 64:65])
                    osq = at.tile([128, 8, DH], BF16, tag="osq")
                    nc.vector.scalar_tensor_tensor(
                        out=osq[0:QB, 0:6, :].rearrange("p (i k) d -> p i k d", k=2),
                        in0=ova[0:QB, :, :, 0:64], scalar=1.0 / W8SC,
                        in1=rsa[0:QB].to_broadcast([QB, 3, 2, 64]),
                        op0=ALU.mult, op1=ALU.mult)
                    nc.vector.scalar_tensor_tensor(
                        out=osq[0:QB, 6:8, :].rearrange("p (i k) d -> p i k d", k=2),
                        in0=ovb[0:QB, :, :, 0:64], scalar=1.0 / W8SC,
                        in1=rsb[0:QB].to_broadcast([QB, 1, 2, 64]),
                        op0=ALU.mult, op1=ALU.mult)
                    # transpose O -> [d, q] (bf16 XBAR), cast fp8 into osT8
                    osT = osTp.tile([128, 4, QB], BF16, tag="osT")
                    eng = nc.sync if b % 2 == 0 else nc.scalar
                    eng.dma_start_transpose(osT[:, :, 0:nq], osq[0:nq, :, :])
                    nc.vector.tensor_copy(out=osT8[:, :, QB * b:QB * b + nq],
                                          in_=osT[:, :, 0:nq])
                emit_outproj(7)
    nc.finalize()
    return nc


def make_in_maps(x, ln_g, ln_b, w_in, b_in, w_out, b_out):
    x = np.asarray(x, np.float32)
    ln_g = np.asarray(ln_g, np.float32)
    ln_b = np.asarray(ln_b, np.float32)
    w_in = np.asarray(w_in, np.float32)
    b_in = np.asarray(b_in, np.float32)
    w_out = np.asarray(w_out, np.float32)
    b_out = np.asarray(b_out, np.float32)

    # fold LN affine into the packed projection (scores descale lives in
    # the exp scale on-device, NOT in the weights)
    w_eff = w_in * ln_g[None, :]
    b_eff = b_in + w_in @ ln_b

    bf = ml_dtypes.bfloat16
    f8 = ml_dtypes.float8_e4m3

    def dr_pack(wT, scale):     # [512, J] -> [128, 2, 2, J] fp8
        J = wT.shape[1]
        return np.ascontiguousarray(
            (wT * scale).reshape(2, 2, 128, J).transpose(2, 0, 1, 3)).astype(f8)

    w8qk = dr_pack(w_eff[:2 * D].T, W8SC)
    w8v = dr_pack(w_eff[2 * D:].T, W8SC)
    wo8 = np.ascontiguousarray(
        (w_out.T * W8SC).reshape(2, 2, 128, D).transpose(0, 2, 1, 3)).astype(f8)
    beffqk = np.ascontiguousarray(
        (b_eff[:2 * D] * W8SC).reshape(8, 128).T).astype(np.float32)
    bo_eff = b_eff[2 * D:] @ w_out.T + b_out      # v-bias folded through Wout

    in_maps = []
    for cidx in range(N_CORES):
        batch = cidx // 4
        t0 = (cidx % 4) * PC
        xloc = np.zeros((STP, D), bf)
        lo = t0 - HALO
        s0, s1 = max(lo, 0), min(t0 + PC + HALO, T)
        xloc[s0 - lo:s1 - lo] = x[batch, s0:s1].astype(bf)
        xr = (x[batch, t0:t0 + PC] + bo_eff[None, :]).astype(np.float32)

        # mask[w, b, q]: band + in-batch bounds (+ block-10 query padding)
        bm = np.zeros((128, NBQ, QB), np.float32)
        ww = np.arange(128)[:, None]
        for b in range(NBQ):
            w0 = w0_of(b)
            qq = np.arange(QB)[None, :]
            keyg = t0 - HALO + w0 + ww                 # [128, 1]
            qg = t0 + QB * b + qq                      # [1, QB]
            m = (np.abs(keyg - qg) <= BAND) & (keyg >= 0) & (keyg < T) \
                & (qg < t0 + PC)
            bm[:, b, :] = m
        in_maps.append(dict(
            xin=xloc, xres=xr, w8qk=w8qk, w8v=w8v, wo8=wo8, beffqk=beffqk,
            bmask=bm.astype(f8)))
    return in_maps


def kernel_run(inputs, trace=False, trace_kwargs=None):
    global _NC_CACHE
    if _NC_CACHE is None:
        _NC_CACHE = build_bass()
    nc = _NC_CACHE
    in_maps = make_in_maps(**inputs)
    kw = {}
    if trace:
        kw = dict(trace=True, trace_cores=[0], **(trace_kwargs or {}))
    res = run_bass_kernel_spmd(nc, in_maps, list(range(N_CORES)), **kw)
    y = np.stack([res.results[c]["yout"] for c in range(N_CORES)])
    out = y.reshape(B, T, D).astype(np.float32)
    return out, res


def kernel(**inputs):
    out, _ = kernel_run(inputs, trace=False)
    return out


if __name__ == "__main__":
    rng = np.random.default_rng(0)
    ins = dict(
        x=rng.standard_normal((B, T, D)).astype(np.float32),
        ln_g=np.ones(D, np.float32), ln_b=np.zeros(D, np.float32),
        w_in=(rng.standard_normal((3 * D, D)) * 0.02).astype(np.float32),
        b_in=np.zeros(3 * D, np.float32),
        w_out=(rng.standard_normal((D, D)) * 0.02).astype(np.float32),
        b_out=np.zeros(D, np.float32))
    out = kernel(**ins)
    print("ran:", out.shape, out.dtype)


# revision 12
# speedup vs baseline: 1.1242x; 1.0434x over previous
"""TRN2 Bass kernel for nn_LocalSelfAttn (LN -> packed QKV -> banded attention
(window +-16) -> out-proj -> residual), sharded 8-way over (B, T):
8 cores x 1024 tokens, 16-token halo strips (zero-padded at sequence edges).

v2 design (vs the 122us baseline):
  - x strip loaded in 3 batched DMAs (HWDGE descriptor-gen is a serial
    ~630ns/instr resource; the old 10 per-tile DMAs trickled in over 16us)
  - PE p-state warmup: a chain of tiny matmuls during LN keeps the PE
    continuously busy so QKV runs at 2.4GHz instead of 1.2GHz
  - QKV projection in fp8e4m3 with perf_mode=DoubleRow (256-deep
    contraction, 0.5 cyc/row): weights prescaled x64 on host, descale
    folded into the exp() scale and the out-proj epilogue
  - ht transposed via 3 batched XBAR DMAs (bf16), then cast to fp8 flat
    layout [d_chunk, tok] so matmul operands can slice arbitrary token
    windows
  - attention in 11 blocks of 96 queries x 128-key windows: no hi/lo
    split matmuls; one exp + one mask-mul per block
  - PV via fp8 DoubleRow pairing two heads per matmul (zero-padded V
    slabs); ones-columns give per-head softmax rowsums for free
  - scores pairs run even/odd head row-groups concurrently; sT PSUM is
    double-buffered (2 banks each) so next-block scores overlap exp
  - out-proj (fp8 DR) emitted as soon as its 128-query osT columns are
    transposed+cast; residual fetched separately from DRAM in f32
    (improves accuracy; off critical path on the gpsimd SWDGE queue)
"""

import sys

for _p in ("/opt/trn_rl_repo",):
    if _p not in sys.path:
        sys.path.insert(0, _p)

import numpy as np
import ml_dtypes

import concourse.bass as bass
import concourse.tile as tile
from concourse import bacc, mybir
from concourse.bass import ts
from concourse.bass_utils import run_bass_kernel_spmd

F32 = mybir.dt.float32
FP8 = mybir.dt.float8e4
BF16 = mybir.dt.bfloat16
AF = mybir.ActivationFunctionType
ALU = mybir.AluOpType
DR = mybir.MatmulPerfMode.DoubleRow

B, T, D, H, BAND = 2, 4096, 512, 8, 16
DH = D // H            # 64
LN_EPS = 1e-5
N_CORES = 8
PC = 1024              # tokens per core
HALO = 16
ST = PC + 2 * HALO     # strip = 1056 real tokens
STP = 1152             # strip padded to 9 full LN tiles
NT = 9                 # LN tiles
QB = 96                # queries per attention block
NBQ = 11               # 10 x 96 + 1 x 64
WIN = 128              # key window per block
NOB = 8                # out-proj blocks of 128 queries
W8SC = 64.0            # fp8 weight prescale
EXPSC = 1.0 / (W8SC * W8SC * np.sqrt(DH))   # descale both x64 + 1/sqrt(dh)
NWARM = 48

# out-proj block ob -> first attention block b whose osT cast completes
# columns [128*ob, 128*ob+128)
OB_DEADLINE = {0: 1, 1: 2, 2: 3, 3: 5, 4: 6, 5: 7, 6: 9, 7: 10}


def w0_of(b):
    return 96 * b if b < 10 else 928


_NC_CACHE = None


def build_bass():
    nc = bacc.Bacc(None, target_bir_lowering=False)
    xin = nc.declare_dram_parameter("xin", [STP, D], BF16, isOutput=False)
    xres = nc.declare_dram_parameter("xres", [PC, D], F32, isOutput=False)
    w8qk = nc.declare_dram_parameter("w8qk", [128, 2, 2, 2 * D], FP8, isOutput=False)
    w8v = nc.declare_dram_parameter("w8v", [128, 2, 2, D], FP8, isOutput=False)
    wo8 = nc.declare_dram_parameter("wo8", [2, 128, 2, D], FP8, isOutput=False)
    beffqk = nc.declare_dram_parameter("beffqk", [128, 8], F32, isOutput=False)
    bmask = nc.declare_dram_parameter("bmask", [128, NBQ, QB], FP8, isOutput=False)
    yout = nc.declare_dram_parameter("yout", [PC, D], F32, isOutput=True)

    with tile.TileContext(nc) as tc:
        from contextlib import ExitStack

        with ExitStack() as ctx:
            const = ctx.enter_context(tc.tile_pool(name="const", bufs=1))
            sb = ctx.enter_context(tc.tile_pool(name="sb", bufs=1))
            ln = ctx.enter_context(tc.tile_pool(name="ln", bufs=4))
            cpq = ctx.enter_context(tc.tile_pool(name="cpq", bufs=2))
            at = ctx.enter_context(tc.tile_pool(name="at", bufs=3))
            osTp = ctx.enter_context(tc.tile_pool(name="osTp", bufs=3))

            # ---- x strip in 3 batched DMAs on sync queue ----
            x_sb = sb.tile([128, NT, D], BF16)
            for g in range(3):
                nc.sync.dma_start(
                    out=x_sb[:, 3 * g:3 * g + 3, :],
                    in_=xin[384 * g:384 * g + 384, :].rearrange(
                        "(t p) d -> p t d", p=128))

            # ---- constants (weights on scalar queue; masks/xres on gpsimd) ----
            beff_sb = const.tile([128, 8], F32)
            nc.scalar.dma_start(out=beff_sb, in_=beffqk[:, :])
            w8_sb = const.tile([128, 2, 2, 2 * D], FP8)
            nc.scalar.dma_start(out=w8_sb, in_=w8qk[:, :, :, :])
            w8v_sb = const.tile([128, 2, 2, D], FP8)
            nc.scalar.dma_start(out=w8v_sb, in_=w8v[:, :, :, :])
            wo_sb = const.tile([128, 2, 2, D], FP8)
            nc.scalar.dma_start(out=wo_sb, in_=wo8.rearrange("c p k j -> p c k j"))
            bm_sb = const.tile([128, NBQ, 1, QB], FP8)
            nc.gpsimd.dma_start(out=bm_sb[:, :, 0, :], in_=bmask[:, :, :])
            xres_sb = const.tile([128, NOB, D], F32)
            for g in range(2):
                nc.gpsimd.dma_start(
                    out=xres_sb[:, 4 * g:4 * g + 4, :],
                    in_=xres[512 * g:512 * g + 512, :].rearrange(
                        "(t p) d -> p t d", p=128))
            eps_sb = const.tile([128, 1], F32)
            nc.vector.memset(eps_sb, LN_EPS)

            # ---- persistent activations ----
            hbf = sb.tile([128, NT, D], BF16)          # normalized h, token-major
            htst = sb.tile([128, 36, 128], BF16)       # h^T staging (tile,dc)-major
            ht8 = sb.tile([128, 4, STP], FP8)          # h^T fp8 flat [dc, tok]
            qk_sb = sb.tile([128, 8, 1088], BF16)      # q,k rows x64, flat tok
            nc.gpsimd.memset(qk_sb[:, 0:4, 1040:1088], 0.0)  # block-10 query pad
            v8 = sb.tile([128, NBQ, 4, 2, 132], FP8)   # per-block DR V slabs
            nc.gpsimd.memset(v8, 0.0)
            nc.gpsimd.memset(v8[:, :, :, 0, 64:65], 1.0)   # rowsum ones cols
            nc.gpsimd.memset(v8[:, :, :, 1, 130:131], 1.0)
            osT8 = sb.tile([128, 4, PC], FP8)          # O^T fp8 [dc, q]

            # ================= Phase A =================
            # PE p-state warmup: chained tiny matmuls during LN keep the PE
            # continuously busy so the first real matmuls run at full clock
            with tc.tile_pool(name="psW", bufs=1, space="PSUM") as psW:
                wt = psW.tile([128, 256], F32)
                for _ in range(NWARM):
                    nc.tensor.matmul(wt[0:2, :], lhsT=x_sb[:, 0, 0:2],
                                     rhs=x_sb[:, 0, 0:256], start=True, stop=True)

            # LayerNorm: stats on vector, sqrt on scalar, normalize split
            for t in range(NT):
                stats = ln.tile([128, 6], F32)
                nc.vector.bn_stats(out=stats, in_=x_sb[:, t, :])
                mv = ln.tile([128, 2], F32)
                nc.vector.bn_aggr(out=mv, in_=stats)
                std = ln.tile([128, 1], F32)
                nc.scalar.activation(out=std, in_=mv[:, 1:2], func=AF.Sqrt,
                                     bias=eps_sb)
                rstd = ln.tile([128, 1], F32)
                nc.vector.reciprocal(out=rstd, in_=std)
                nbias = ln.tile([128, 1], F32)
                nc.vector.tensor_scalar(
                    out=nbias, in0=mv[:, 0:1], scalar1=rstd, scalar2=-1.0,
                    op0=ALU.mult, op1=ALU.mult)
                if t % 2 == 0:
                    nc.scalar.activation(out=hbf[:, t, :], in_=x_sb[:, t, :],
                                         func=AF.Identity, bias=nbias, scale=rstd)
                else:
                    nc.vector.tensor_scalar(
                        out=hbf[:, t, :], in0=x_sb[:, t, :], scalar1=rstd,
                        scalar2=nbias, op0=ALU.mult, op1=ALU.add)
                if t % 3 == 2:
                    g = t // 3
                    nc.sync.dma_start_transpose(
                        htst[:, 12 * g:12 * g + 12, :], hbf[:, 3 * g:3 * g + 3, :])
                    # cast to fp8 + permute (tile,dc)-major -> flat [dc, tok]
                    # (gpsimd handles fp8 ~10x slower than DVE/Act: keep off it)
                    dst = ht8[:, :, 384 * g:384 * g + 384].rearrange(
                        "p c (t x) -> p t c x", x=128)
                    src = htst[:, 12 * g:12 * g + 12, :].rearrange(
                        "p (t c) x -> p t c x", c=4)
                    if g == 1:
                        nc.scalar.activation(out=dst, in_=src, func=AF.Identity)
                    else:
                        nc.vector.tensor_copy(out=dst, in_=src)

            with tc.tile_pool(name="psA", bufs=2, space="PSUM") as psA:
                # V projection for per-block 128-token key windows (fp8 DR).
                # Emits the DR rhs slabs [w, 2, 132] with head pairs zero-
                # padded; data cols [66k, 66k+64), ones col at 66k+64.
                def emit_vproj(b):
                    w0 = w0_of(b)
                    vp = psA.tile([128, 4, 2, 64], F32, tag="v", name="vp")
                    for c in range(2):
                        nc.tensor.matmul(
                            vp[:, :, :, :], lhsT=ht8[:, 2 * c:2 * c + 2, w0:w0 + 128],
                            rhs=w8v_sb[:, c, :, :], start=(c == 0), stop=(c == 1),
                            perf_mode=DR)
                    nc.vector.tensor_copy(
                        out=v8[:, b, :, 0, 0:64], in_=vp[:, :, 0, :])
                    nc.scalar.activation(
                        out=v8[:, b, :, 1, 66:130], in_=vp[:, :, 1, :],
                        func=AF.Identity)

                emit_vproj(0)
                emit_vproj(1)

                # Q (jc 0-3, tokens [16,1040)) and K (jc 4-7, tokens [0,1056))
                QCH = [(16, 528), (528, 1040)]
                KCH = [(0, 512), (512, 1024), (1024, 1056)]
                vnext = 2
                for jc in range(8):
                    t_lo, chunks = (16, QCH) if jc < 4 else (0, KCH)
                    qp = psA.tile([128, 1056], F32, tag="qk", name="qp")
                    for (s0, s1) in chunks:
                        for c in range(2):
                            nc.tensor.matmul(
                                qp[:, s0 - t_lo:s1 - t_lo],
                                lhsT=w8_sb[:, c, :, ts(jc, 128)],
                                rhs=ht8[:, 2 * c:2 * c + 2, s0:s1],
                                start=(c == 0), stop=(c == 1), perf_mode=DR)
                    if vnext < NBQ:
                        emit_vproj(vnext)
                        vnext += 1
                    # PSUM -> SBUF bf16 with (x64-prescaled) bias add; split
                    # in two halves so early attention blocks unblock before
                    # the full token range is evacuated
                    width = chunks[-1][1] - t_lo
                    for (h0, h1) in ((0, 512), (512, width)):
                        if jc % 2 == 0:
                            nc.scalar.activation(
                                out=qk_sb[:, jc, t_lo + h0:t_lo + h1],
                                in_=qp[:, h0:h1], func=AF.Identity,
                                bias=beff_sb[:, jc:jc + 1])
                        else:
                            nc.vector.tensor_scalar_add(
                                out=qk_sb[:, jc, t_lo + h0:t_lo + h1],
                                in0=qp[:, h0:h1], scalar1=beff_sb[:, jc:jc + 1])
                while vnext < NBQ:
                    emit_vproj(vnext)
                    vnext += 1

            # ================= Phase B: attention =================
            with (
                tc.tile_pool(name="psS", bufs=2, space="PSUM") as psS,
                tc.tile_pool(name="psO", bufs=1, space="PSUM") as psO,
                tc.tile_pool(name="psY", bufs=2, space="PSUM") as psY,
            ):
                def emit_outproj(ob):
                    yp = psY.tile([128, D], F32, tag="yp", name="yp")
                    for c in range(2):
                        nc.tensor.matmul(
                            yp, lhsT=osT8[:, 2 * c:2 * c + 2, ts(ob, 128)],
                            rhs=wo_sb[:, c, :, :], start=(c == 0), stop=(c == 1),
                            perf_mode=DR)
                    ysb = at.tile([128, D], F32, tag="ysb", name="ysb")
                    nc.vector.scalar_tensor_tensor(
                        out=ysb, in0=yp, scalar=1.0 / W8SC,
                        in1=xres_sb[:, ob, :], op0=ALU.mult, op1=ALU.add)
                    nc.gpsimd.dma_start(out=yout[ts(ob, 128), :], in_=ysb)

                emit_at = {dl + 1: ob for ob, dl in OB_DEADLINE.items() if dl < 10}

                for b in range(NBQ):
                    q0 = HALO + QB * b
                    w0 = w0_of(b)
                    nq = QB if b < 10 else 64
                    # scores S^T[w, q]; head 2i+k -> slot 4k+i; even/odd head
                    # row-groups (partitions 0-63 / 64-127) run concurrently
                    # and land in different PSUM banks
                    sT = psS.tile([128, 8, 128], F32, tag="sT")
                    for i in range(4):
                        for k in range(2):
                            p64 = 64 * k
                            nc.tensor.matmul(
                                sT[:, 4 * k + i, 0:QB],
                                lhsT=qk_sb[p64:p64 + 64, 4 + i, w0:w0 + WIN],
                                rhs=qk_sb[p64:p64 + 64, i, q0:q0 + QB],
                                start=True, stop=True)
                    # exp (with all descales folded into scale) -> fp8 P
                    pth = at.tile([128, 4, 2, QB], FP8, tag="pth")
                    nc.scalar.activation(
                        out=pth[:, :, :, :].rearrange("p i k q -> p k i q"),
                        in_=sT[:, :, 0:QB].rearrange("p (k i) q -> p k i q", k=2),
                        func=AF.Exp, scale=float(EXPSC))
                    nc.vector.tensor_mul(
                        out=pth[:, :, :, :], in0=pth[:, :, :, :],
                        in1=bm_sb[:, b:b + 1, :, :].to_broadcast([128, 4, 2, QB]))
                    if b in emit_at:
                        emit_outproj(emit_at[b])
                    # PV: fp8 DR, two heads per matmul via zero-padded slabs
                    ova = psO.tile([128, 3, 2, 66], F32, tag="ova")
                    ovb = psO.tile([128, 1, 2, 66], F32, tag="ovb")
                    for i in range(4):
                        out = ova[0:QB, i, :, :] if i < 3 else ovb[0:QB, 0, :, :]
                        nc.tensor.matmul(
                            out, lhsT=pth[:, i, :, :], rhs=v8[:, b, i, :, :],
                            start=True, stop=True, perf_mode=DR)
                    # normalize: rowsums at col 64 of each slab
                    rsa = at.tile([128, 3, 2, 1], F32, tag="rsa")
                    nc.vector.reciprocal(out=rsa[0:QB], in_=ova[0:QB, :, :, 64:65])
                    rsb = at.tile([128, 1, 2, 1], F32, tag="rsb")
                    nc.vector.reciprocal(out=rsb[0:QB], in_=ovb[0:QB, :, :, 64:65])
                    osq = at.tile([128, 8, DH], BF16, tag="osq")
                    nc.vector.scalar_tensor_tensor(
                        out=osq[0:QB, 0:6, :].rearrange("p (i k) d -> p i k d", k=2),
                        in0=ova[0:QB, :, :, 0:64], scalar=1.0 / W8SC,
                        in1=rsa[0:QB].to_broadcast([QB, 3, 2, 64]),
                        op0=ALU.mult, op1=ALU.mult)
                    nc.vector.scalar_tensor_tensor(
                        out=osq[0:QB, 6:8, :].rearrange("p (i k) d -> p i k d", k=2),
                        in0=ovb[0:QB, :, :, 0:64], scalar=1.0 / W8SC,
                        in1=rsb[0:QB].to_broadcast([QB, 1, 2, 64]),
                        op0=ALU.mult, op1=ALU.mult)
                    # transpose O -> [d, q] (bf16 XBAR), cast fp8 into osT8
                    osT = osTp.tile([128, 4, QB], BF16, tag="osT")
                    nc.sync.dma_start_transpose(osT[:, :, 0:nq], osq[0:nq, :, :])
                    nc.scalar.activation(out=osT8[:, :, QB * b:QB * b + nq],
                                         in_=osT[:, :, 0:nq], func=AF.Identity)
                emit_outproj(7)
    nc.finalize()
    return nc


def make_in_maps(x, ln_g, ln_b, w_in, b_in, w_out, b_out):
    x = np.asarray(x, np.float32)
    ln_g = np.asarray(ln_g, np.float32)
    ln_b = np.asarray(ln_b, np.float32)
    w_in = np.asarray(w_in, np.float32)
    b_in = np.asarray(b_in, np.float32)
    w_out = np.asarray(w_out, np.float32)
    b_out = np.asarray(b_out, np.float32)

    # fold LN affine into the packed projection (scores descale lives in
    # the exp scale on-device, NOT in the weights)
    w_eff = w_in * ln_g[None, :]
    b_eff = b_in + w_in @ ln_b

    bf = ml_dtypes.bfloat16
    f8 = ml_dtypes.float8_e4m3

    def dr_pack(wT, scale):     # [512, J] -> [128, 2, 2, J] fp8
        J = wT.shape[1]
        return np.ascontiguousarray(
            (wT * scale).reshape(2, 2, 128, J).transpose(2, 0, 1, 3)).astype(f8)

    w8qk = dr_pack(w_eff[:2 * D].T, W8SC)
    w8v = dr_pack(w_eff[2 * D:].T, W8SC)
    wo8 = np.ascontiguousarray(
        (w_out.T * W8SC).reshape(2, 2, 128, D).transpose(0, 2, 1, 3)).astype(f8)
    beffqk = np.ascontiguousarray(
        (b_eff[:2 * D] * W8SC).reshape(8, 128).T).astype(np.float32)
    bo_eff = b_eff[2 * D:] @ w_out.T + b_out      # v-bias folded through Wout

    in_maps = []
    for cidx in range(N_CORES):
        batch = cidx // 4
        t0 = (cidx % 4) * PC
        xloc = np.zeros((STP, D), bf)
        lo = t0 - HALO
        s0, s1 = max(lo, 0), min(t0 + PC + HALO, T)
        xloc[s0 - lo:s1 - lo] = x[batch, s0:s1].astype(bf)
        xr = (x[batch, t0:t0 + PC] + bo_eff[None, :]).astype(np.float32)

        # mask[w, b, q]: band + in-batch bounds (+ block-10 query padding)
        bm = np.zeros((128, NBQ, QB), np.float32)
        ww = np.arange(128)[:, None]
        for b in range(NBQ):
            w0 = w0_of(b)
            qq = np.arange(QB)[None, :]
            keyg = t0 - HALO + w0 + ww                 # [128, 1]
            qg = t0 + QB * b + qq                      # [1, QB]
            m = (np.abs(keyg - qg) <= BAND) & (keyg >= 0) & (keyg < T) \
                & (qg < t0 + PC)
            bm[:, b, :] = m
        in_maps.append(dict(
            xin=xloc, xres=xr, w8qk=w8qk, w8v=w8v, wo8=wo8, beffqk=beffqk,
            bmask=bm.astype(f8)))
    return in_maps


def kernel_run(inputs, trace=False, trace_kwargs=None):
    global _NC_CACHE
    if _NC_CACHE is None:
        _NC_CACHE = build_bass()
    nc = _NC_CACHE
    in_maps = make_in_maps(**inputs)
    kw = {}
    if trace:
        kw = dict(trace=True, trace_cores=[0], **(trace_kwargs or {}))
    res = run_bass_kernel_spmd(nc, in_maps, list(range(N_CORES)), **kw)
    y = np.stack([res.results[c]["yout"] for c in range(N_CORES)])
    out = y.reshape(B, T, D).astype(np.float32)
    return out, res


def kernel(**inputs):
    out, _ = kernel_run(inputs, trace=False)
    return out


if __name__ == "__main__":
    rng = np.random.default_rng(0)
    ins = dict(
        x=rng.standard_normal((B, T, D)).astype(np.float32),
        ln_g=np.ones(D, np.float32), ln_b=np.zeros(D, np.float32),
        w_in=(rng.standard_normal((3 * D, D)) * 0.02).astype(np.float32),
        b_in=np.zeros(3 * D, np.float32),
        w_out=(rng.standard_normal((D, D)) * 0.02).astype(np.float32),
        b_out=np.zeros(D, np.float32))
    out = kernel(**ins)
    print("ran:", out.shape, out.dtype)
